# revision 4
# baseline (speedup 1.0000x reference)
"""Causal multi-head self-attention with RoPE on 8 Trainium2 NeuronCores.

Sharding: core = (batch b, head-group g) with b = core//2, g = core%2.
Each core computes QKV projections for its batch element restricted to its
8 heads (512 of 1024 projection rows), RoPE, causal attention, and the
partial output projection y_g = attn_g @ wo[:, g*512:(g+1)*512].T.  The host
sums the two head-group partials per batch element.

Key layout trick: scores are computed TRANSPOSED (k on partitions, q on the
free axis), so softmax needs no free-axis reduction at all: exp() is
elementwise, the denominator falls out of the PV matmul via a ones column
appended to V, and exp(scores^T) feeds the PV matmul directly as the moving
operand (no transpose of the probabilities is ever needed).

RoPE pairing (even, odd) is turned into contiguous [evens | odds] blocks by
permuting the rows of wq/wk on the host (cancels in q.k^T), so the rotation
is 6 full-width strided vector ops per tile instead of lane-starved ops.

mm dtype modes (env KMODE): "f32" (4 cyc/row), "f32r" (1 cyc/row, reduced
precision), "bf16" (1 cyc/row, operands cast on host / on eviction).
"""
import math
import os
from contextlib import ExitStack

import numpy as np
import ml_dtypes

import concourse.bass as bass
import concourse.tile as tile
from concourse import bacc, mybir
from concourse import masks
from concourse.bass_utils import run_bass_kernel_spmd

F32 = mybir.dt.float32
BF16 = mybir.dt.bfloat16

D = 1024          # d_model
NH = 16           # heads total
DK = 64           # head dim
S = 2048          # sequence
B = 4             # batch
THETA = 10000.0
HPG = 8           # heads per group (2 groups over 8 cores with 4 batches)
W = HPG * DK      # 512: local projection width
NSB = S // 128    # 16 s-blocks
NQG = 4           # 512-wide q groups
NEG = -1.0e10     # additive causal mask value

MODE = os.environ.get("KMODE", "f32")     # "f32" | "f32r" | "bf16"
TRACE = bool(int(os.environ.get("KTRACE", "0")))

_cache = {}


def _mmcast(ap):
    if MODE == "f32r":
        return ap.bitcast(mybir.dt.float32r)
    return ap


def build_nc():
    sdt = BF16 if MODE == "bf16" else F32
    nc = bacc.Bacc(None, target_bir_lowering=False, debug=False)

    xt = nc.dram_tensor("xt", [D, S], sdt, kind="ExternalInput")
    wqt = nc.dram_tensor("wqt", [D, W], sdt, kind="ExternalInput")
    wkt = nc.dram_tensor("wkt", [D, W], sdt, kind="ExternalInput")
    wvt = nc.dram_tensor("wvt", [D, W], sdt, kind="ExternalInput")
    wot = nc.dram_tensor("wot", [W, D], sdt, kind="ExternalInput")
    cosb = nc.dram_tensor("cosb", [S, W // 2], F32, kind="ExternalInput")
    sinb = nc.dram_tensor("sinb", [S, W // 2], F32, kind="ExternalInput")
    yp = nc.dram_tensor("yp", [S, D], F32, kind="ExternalOutput")

    xt3 = xt[:].rearrange("(jo p) s -> p jo s", p=128)       # [128, 8, S]
    wqt3 = wqt[:].rearrange("(jo p) i -> p jo i", p=128)     # [128, 8, W]
    wkt3 = wkt[:].rearrange("(jo p) i -> p jo i", p=128)
    wvt3 = wvt[:].rearrange("(jo p) i -> p jo i", p=128)
    wot3 = wot[:].rearrange("(jo p) i -> p jo i", p=128)     # [128, 4, D]

    with tile.TileContext(nc) as tc, ExitStack() as ctx:
        persist = ctx.enter_context(tc.tile_pool(name="persist", bufs=1))
        ident = persist.tile([128, 128], sdt, name="ident")
        masks.make_identity(nc, ident)
        # additive causal mask for the transposed diagonal block:
        # tri[k, q] = 0 where q >= k else NEG
        tri = persist.tile([128, 128], F32, name="tri")
        nc.gpsimd.memset(tri, 0.0)
        nc.gpsimd.affine_select(
            out=tri, in_=tri, compare_op=mybir.AluOpType.is_ge, fill=NEG,
            base=0, pattern=[[1, 128]], channel_multiplier=-1)

        # persistent activations: q^T and k^T as 4 head-pair slabs
        # (rows = the 128 dims of heads (2j, 2j+1)), v s-major with a ones
        # column per head ([... v_h (64) | 1 ...] -> 65 cols per head).
        qT = [persist.tile([128, S], sdt, name=f"qT{j}") for j in range(4)]
        kT = [persist.tile([128, S], sdt, name=f"kT{j}") for j in range(4)]
        vt = [persist.tile([128, HPG * (DK + 1)], sdt, name=f"vt{i}")
              for i in range(NSB)]

        # ---------------- phase 1: projections + RoPE + transposes --------
        with tc.tile_pool(name="wp", bufs=1) as wp, \
             tc.tile_pool(name="p1t", bufs=3) as p1t, \
             tc.tile_pool(name="p1p", bufs=3, space="PSUM") as p1p, \
             tc.tile_pool(name="p1tr", bufs=4, space="PSUM") as p1tr:
            wq_s = wp.tile([128, 8, W], sdt, name="wq_s")
            nc.sync.dma_start(wq_s[:], wqt3[:])
            wk_s = wp.tile([128, 8, W], sdt, name="wk_s")
            nc.sync.dma_start(wk_s[:], wkt3[:])
            wv_s = wp.tile([128, 8, W], sdt, name="wv_s")
            nc.sync.dma_start(wv_s[:], wvt3[:])

            def rope(ps, outt, c3, s3):
                # ps: [128, W] PSUM (pre-RoPE proj, s-major, heads as
                # [evens(32) | odds(32)] blocks); outt: [128, W] SBUF
                pe = ps.rearrange("p (h eo c) -> p h eo c", eo=2, c=32)
                ein, oin = pe[:, :, 0, :], pe[:, :, 1, :]
                oe = outt.rearrange("p (h eo c) -> p h eo c", eo=2, c=32)
                eout, oout = oe[:, :, 0, :], oe[:, :, 1, :]
                ra = p1t.tile([128, 8, 32], F32, name="ra", tag="ra")
                rb = p1t.tile([128, 8, 32], F32, name="rb", tag="rb")
                nc.vector.tensor_mul(ra, ein, c3)
                nc.vector.tensor_mul(rb, oin, s3)
                nc.vector.tensor_sub(eout, ra, rb)
                rc = p1t.tile([128, 8, 32], F32, name="rc", tag="rc")
                rd = p1t.tile([128, 8, 32], F32, name="rd", tag="rd")
                nc.vector.tensor_mul(rc, ein, s3)
                nc.vector.tensor_mul(rd, oin, c3)
                nc.vector.tensor_add(oout, rc, rd)

            for sb in range(NSB):
                s0 = sb * 128
                xs = p1t.tile([128, 8, 128], sdt, name="xs", tag="xs")
                nc.sync.dma_start(xs[:], xt3[:, :, s0:s0 + 128])
                cs = p1t.tile([128, W // 2], F32, name="cs", tag="cs")
                nc.sync.dma_start(cs[:], cosb[s0:s0 + 128, :])
                sn = p1t.tile([128, W // 2], F32, name="sn", tag="sn")
                nc.sync.dma_start(sn[:], sinb[s0:s0 + 128, :])
                c3 = cs.rearrange("p (h c) -> p h c", c=32)
                s3 = sn.rearrange("p (h c) -> p h c", c=32)

                pq = p1p.tile([128, W], F32, name="pq", tag="pp")
                pk = p1p.tile([128, W], F32, name="pk", tag="pp")
                pv = p1p.tile([128, W], F32, name="pv", tag="pp")
                for dst, wsb in ((pq, wq_s), (pk, wk_s), (pv, wv_s)):
                    for jo in range(8):
                        nc.tensor.matmul(
                            dst[:], _mmcast(xs[:, jo, :]),
                            _mmcast(wsb[:, jo, :]),
                            start=(jo == 0), stop=(jo == 7))

                q_ro = p1t.tile([128, W], sdt, name="q_ro", tag="qro")
                rope(pq, q_ro, c3, s3)
                k_ro = p1t.tile([128, W], sdt, name="k_ro", tag="kro")
                rope(pk, k_ro, c3, s3)

                # v eviction (+ ones column per head)
                v3 = vt[sb].rearrange("p (h c) -> p h c", c=DK + 1)
                nc.scalar.copy(v3[:, :, 0:DK],
                               pv.rearrange("p (h c) -> p h c", c=DK))
                nc.gpsimd.memset(v3[:, :, DK:DK + 1], 1.0)

                # per-head-pair transposes into the d-major slabs
                for pr in range(4):
                    c0 = pr * 128
                    for src, dstl in ((q_ro, qT), (k_ro, kT)):
                        ptr = p1tr.tile([128, 128], F32, name="ptr", tag="tr")
                        nc.tensor.transpose(
                            ptr[:], _mmcast(src[:, c0:c0 + 128]),
                            _mmcast(ident[:]))
                        nc.scalar.copy(dstl[pr][:, s0:s0 + 128], ptr[:])

        # ------------- phase 2+3: attention + output projection -----------
        with tc.tile_pool(name="p2c", bufs=1) as p2c, \
             tc.tile_pool(name="p2t", bufs=3) as p2t, \
             tc.tile_pool(name="exp", bufs=4) as expp, \
             tc.tile_pool(name="scp", bufs=3, space="PSUM") as scp, \
             tc.tile_pool(name="pvp", bufs=2, space="PSUM") as pvp, \
             tc.tile_pool(name="pyp", bufs=2, space="PSUM") as pyp:
            wo_s = p2c.tile([128, 4, D], sdt, name="wo_s")
            nc.sync.dma_start(wo_s[:], wot3[:])

            for qg in range(NQG):
                q0 = qg * 512
                aq = [p2t.tile([128, 512], sdt, name=f"aq{j}", tag=f"aq{j}",
                               bufs=2) for j in range(4)]
                for h in range(HPG):
                    slab, r0 = h // 2, 64 * (h % 2)
                    pv = pvp.tile([DK + 1, 512], F32, name="pvh", tag="pv")
                    nkb = 4 * qg + 4
                    for kb in range(nkb):
                        off = kb - 4 * qg           # >= 0 on diagonal blocks
                        c0 = 128 * max(off, 0)
                        wd = 512 - c0
                        sc = scp.tile([128, 512], F32, name="sc", tag="sc")
                        nc.tensor.matmul(
                            sc[:, c0:512],
                            _mmcast(kT[slab][r0:r0 + DK,
                                             kb * 128:(kb + 1) * 128]),
                            _mmcast(qT[slab][r0:r0 + DK, q0 + c0:q0 + 512]),
                            start=True, stop=True)
                        if off >= 0:
                            nc.vector.tensor_add(
                                sc[:, c0:c0 + 128], sc[:, c0:c0 + 128], tri)
                        ex = expp.tile([128, 512], sdt, name="ex", tag="ex")
                        nc.scalar.activation(
                            ex[:, 0:wd], sc[:, c0:512],
                            mybir.ActivationFunctionType.Exp,
                            scale=1.0 / math.sqrt(DK))
                        nc.tensor.matmul(
                            pv[:, c0:512],
                            _mmcast(vt[kb][:, h * (DK + 1):
                                           (h + 1) * (DK + 1)]),
                            _mmcast(ex[:, 0:wd]),
                            start=(kb == 0), stop=(kb == nkb - 1))
                    # normalize: attn^T = pv[0:64] * (1/denom) broadcast
                    rsb = p2t.tile([1, 512], F32, name="rsb", tag="rsb")
                    # NB: custom-DVE reciprocal_approx_* returns garbage on HW
                    # in this environment; the native InstReciprocal works.
                    nc.vector.reciprocal(rsb[:], pv[DK:DK + 1, :])
                    rbc = p2t.tile([64, 512], F32, name="rbc", tag="rbc")
                    nc.gpsimd.partition_broadcast(rbc[:], rsb[:], channels=64)
                    nc.vector.tensor_mul(
                        aq[slab][r0:r0 + 64, :], pv[0:DK, :], rbc[:])
                # output projection for this q-group's 4 s-blocks
                for sbl in range(4):
                    s0 = q0 + sbl * 128
                    yt = p2t.tile([128, D], F32, name="yt", tag="yt")
                    for ih in range(2):
                        py = pyp.tile([128, 512], F32, name="py", tag="py")
                        for j in range(4):
                            nc.tensor.matmul(
                                py[:],
                                _mmcast(aq[j][:, sbl * 128:(sbl + 1) * 128]),
                                _mmcast(wo_s[:, j, ih * 512:(ih + 1) * 512]),
                                start=(j == 0), stop=(j == 3))
                        nc.scalar.copy(yt[:, ih * 512:(ih + 1) * 512], py[:])
                    nc.sync.dma_start(yp[s0:s0 + 128, :], yt[:])

    nc.compile()
    return nc


def _host_dtype():
    return ml_dtypes.bfloat16 if MODE == "bf16" else np.float32


def _prep_inputs(x, token_positions, wq, wk, wv, wo):
    hdt = _host_dtype()
    # per-head permutation: [0,2,...,62, 1,3,...,63] (evens then odds)
    pi = np.concatenate([np.arange(0, DK, 2), np.arange(1, DK, 2)])
    perm = (np.arange(NH)[:, None] * DK + pi[None, :]).reshape(-1)
    wq_p = wq[perm, :]
    wk_p = wk[perm, :]

    pos = np.asarray(token_positions).astype(np.float32)
    thetas = (1.0 / (THETA ** (2.0 * np.arange(DK // 2, dtype=np.float32)
                               / DK))).astype(np.float32)
    ang = np.outer(pos, thetas).astype(np.float32)          # [S, 32]
    cos = np.tile(np.cos(ang), (1, HPG)).astype(np.float32)  # [S, 256]
    sin = np.tile(np.sin(ang), (1, HPG)).astype(np.float32)

    in_maps = []
    for core in range(8):
        b, g = core // 2, core % 2
        gs = slice(g * W, (g + 1) * W)
        in_maps.append({
            "xt": np.ascontiguousarray(x[b].T).astype(hdt),
            "wqt": np.ascontiguousarray(wq_p[gs, :].T).astype(hdt),
            "wkt": np.ascontiguousarray(wk_p[gs, :].T).astype(hdt),
            "wvt": np.ascontiguousarray(wv[gs, :].T).astype(hdt),
            "wot": np.ascontiguousarray(wo[:, gs].T).astype(hdt),
            "cosb": cos,
            "sinb": sin,
        })
    return in_maps


last_exec_time_ns = None


def _install_ntff_hook_shim():
    """This image's antenv lacks axon_hooks; wire the ctypes NTFF hook from
    trn_agent_boot so trace=True yields HW exec times."""
    import sys as _sys
    import types as _types
    try:
        from antenv import axon_hooks  # noqa: F401
        return
    except ImportError:
        pass
    from trn_agent_boot.trn_boot import _ntff_profile_via_ctypes
    hook = _ntff_profile_via_ctypes("/opt/axon/libaxon_pjrt.so")
    mod = _types.ModuleType("antenv.axon_hooks")
    mod.get_axon_ntff_profile_hook = lambda: hook
    _sys.modules["antenv.axon_hooks"] = mod


def kernel(x, token_positions, wq, wk, wv, wo):
    global last_exec_time_ns
    x = np.asarray(x, dtype=np.float32)
    token_positions = np.asarray(token_positions)
    wq = np.asarray(wq, dtype=np.float32)
    wk = np.asarray(wk, dtype=np.float32)
    wv = np.asarray(wv, dtype=np.float32)
    wo = np.asarray(wo, dtype=np.float32)

    if "nc" not in _cache:
        _cache["nc"] = build_nc()
    nc = _cache["nc"]

    in_maps = _prep_inputs(x, token_positions, wq, wk, wv, wo)
    res = None
    if TRACE:
        try:
            _install_ntff_hook_shim()
            res = run_bass_kernel_spmd(nc, in_maps, list(range(8)),
                                       trace=True,
                                       trace_cores=list(range(8)))
        except Exception as e:  # profiling must never sink correctness
            print(f"trace run failed ({type(e).__name__}: {e}); "
                  f"retrying untraced")
            res = None
    if res is None:
        res = run_bass_kernel_spmd(nc, in_maps, list(range(8)))
    last_exec_time_ns = res.exec_time_ns

    out = np.empty((B, S, D), dtype=np.float32)
    for b in range(B):
        out[b] = res.results[2 * b]["yp"] + res.results[2 * b + 1]["yp"]
    return out
